# revision 8
# baseline (speedup 1.0000x reference)
"""Trainium2 Bass kernel for nn_MultiHeadAttention (B=2, S=2048, H=16, d_model=1024).

Sharding (8 cores): data-parallel over batch (2) x tensor-parallel over heads
(4 heads per core, Megatron-style column/row split of the Q/K/V/O projections).
Each core computes a partial output [S, d_model] for its batch; the host sums
the 4 partials per batch and adds the output bias.

Per-core pipeline (all matmuls in float32r = full-speed TF32-grade):
  - stream 4 tq-chunks of 512 tokens; per chunk project q/k (transposed
    layout [e, t]) and v ([t, e] with a fused ones-column per head so the
    softmax denominator falls out of the ctx matmul's 65th row)
  - causal flash-style attention in s^T layout [tk, tq]: row-packed K=64
    score matmuls (2 heads concurrently), exp on ScalarE (PSUM->SBUF),
    causal masking of diagonal blocks via in-place gpsimd affine_select,
    ctx^T accumulation with M=65 matmuls; strictly-upper blocks skipped.
    Score/ctx matmuls are grouped 4 tk-tiles at a time to limit PE
    tiling-mode switches (K=64 row-tiled mode vs K=128 full mode).
  - softmax denominators are DMA-transposed to [128, 8] so the Newton
    reciprocal runs across lanes, then broadcast back across partitions
  - output projection row-packed over the two head-pairs (K=128 each)
"""
import sys

for _p in ("/opt/trn_rl_repo", "/root/.axon_site/_ro/trn_rl_repo"):
    if _p not in sys.path:
        sys.path.insert(0, _p)

import numpy as np

import concourse.bass as bass  # noqa: F401
import concourse.mybir as mybir
from concourse import bacc
from concourse.tile import TileContext
from concourse.tile import add_dep_helper
from concourse.bass_utils import run_bass_kernel_spmd

H = 16
D_MODEL = 1024
D_K = 64
B, S = 2, 2048
N_CORES = 8
HEADS_PER_CORE = 4
E = HEADS_PER_CORE * D_K  # 256 output channels per core
CH = 512                  # tq chunk width
N_CH = S // CH            # 4 chunks
N_TB = S // 128           # 16 token blocks

F32 = mybir.dt.float32
F32R = mybir.dt.float32r
EXP = mybir.ActivationFunctionType.Exp

_NC_CACHE = None


def build_nc():
    nc = bacc.Bacc("TRN2", target_bir_lowering=False, debug=False,
                   enable_asserts=False)
    # x tensors host-packed as [p, chunk, kd, t] so each chunk DMA is 128
    # contiguous 16KB rows
    xq = nc.dram_tensor("xq", (128, N_CH, 8, CH), F32R, kind="ExternalInput").ap()
    xk = nc.dram_tensor("xk", (128, N_CH, 8, CH), F32R, kind="ExternalInput").ap()
    xv = nc.dram_tensor("xv", (128, N_CH, 8, CH), F32R, kind="ExternalInput").ap()
    wq = nc.dram_tensor("wq", (128, 8, E), F32R, kind="ExternalInput").ap()
    wk = nc.dram_tensor("wk", (128, 8, E), F32R, kind="ExternalInput").ap()
    wv = nc.dram_tensor("wv", (128, 8, E), F32R, kind="ExternalInput").ap()
    wo = nc.dram_tensor("wo", (128, 2, D_MODEL), F32R, kind="ExternalInput").ap()
    bq = nc.dram_tensor("bq", (128, 2), F32, kind="ExternalInput").ap()
    bk = nc.dram_tensor("bk", (128, 2), F32, kind="ExternalInput").ap()
    bv = nc.dram_tensor("bv", (1, E), F32, kind="ExternalInput").ap()
    part = nc.dram_tensor("part", (S, D_MODEL), F32, kind="ExternalOutput").ap()

    with TileContext(nc) as tc:
        with tc.tile_pool(name="const", bufs=1) as cp, \
             tc.tile_pool(name="xc", bufs=6) as xcp, \
             tc.tile_pool(name="wk_", bufs=3) as wkp, \
             tc.tile_pool(name="pp", bufs=2, space="PSUM") as ppp, \
             tc.tile_pool(name="etp", bufs=2, space="PSUM") as etpp, \
             tc.tile_pool(name="ctxp", bufs=1, space="PSUM") as ctxp:

            # ---- one-time loads; tiny/bias/mask work first so the gpsimd
            # library reload happens during the DMA head ------------------
            bq_sb = cp.tile([128, 2], F32, tag="bq_sb")
            bk_sb = cp.tile([128, 2], F32, tag="bk_sb")
            bv_sb = cp.tile([1, E], F32, tag="bv_sb")
            nc.sync.dma_start(bq_sb[:], bq[:])
            nc.sync.dma_start(bk_sb[:], bk[:])
            nc.sync.dma_start(bv_sb[:], bv[:])
            bvb = cp.tile([128, E], F32, tag="bvb")
            nc.gpsimd.partition_broadcast(bvb[:], bv_sb[:], channels=128)

            # causal masks for diagonal blocks: keep iff f - p - r*128 >= 0
            masks = cp.tile([128, 4, CH], F32R, tag="masks")
            nc.vector.memset(masks[:].bitcast(F32), 1.0)
            for r in range(4):
                nc.gpsimd.affine_select(
                    out=masks[:, r, :], in_=masks[:, r, :],
                    pattern=[[1, CH]], base=-r * 128,
                    channel_multiplier=-1,
                    compare_op=mybir.AluOpType.is_ge, fill=0.0)

            wq_sb = cp.tile([128, 8, E], F32R, tag="wq_sb")
            wk_sb = cp.tile([128, 8, E], F32R, tag="wk_sb")
            wv_sb = cp.tile([128, 8, E], F32R, tag="wv_sb")
            wo_sb = cp.tile([128, 2, D_MODEL], F32R, tag="wo_sb")
            for kd in range(8):
                nc.sync.dma_start(wq_sb[:, kd, :], wq[:, kd, :])

            # persistent activations (f32r)
            qT = [cp.tile([128, S], F32R, tag=f"qT{p}", name=f"qT{p}")
                  for p in range(2)]
            kT = [cp.tile([128, S], F32R, tag=f"kT{p}", name=f"kT{p}")
                  for p in range(2)]
            va = [cp.tile([128, N_TB, 130], F32R, tag=f"va{p}", name=f"va{p}")
                  for p in range(2)]
            ctxT = [cp.tile([128, S], F32R, tag=f"ctxT{p}", name=f"ctxT{p}")
                    for p in range(2)]
            for p in range(2):
                nc.vector.memset(va[p][:, :, 64:65].bitcast(F32), 1.0)
                nc.vector.memset(va[p][:, :, 129:130].bitcast(F32), 1.0)

            # ---- main chunk loop ------------------------------------------
            def load_xc(src, c, gate):
                # two half tiles for finer prefetch rotation
                halves = []
                for half in range(2):
                    xh = xcp.tile([128, 4, CH], F32R, tag="xc", name="xc")
                    for kd in range(4):
                        d = nc.sync.dma_start(xh[:, kd, :],
                                              src[:, c, 4 * half + kd, :])
                        if gate is not None:
                            add_dep_helper(d.ins, gate.ins, sync=True,
                                           reason="dma-throttle")
                    halves.append(xh)
                return lambda kd: halves[kd // 4][:, kd % 4, :]

            def emit_proj(c):
                csl = slice(c * CH, (c + 1) * CH)
                gates = {}
                # q/k projections -> qT/kT[e, t-chunk]; on chunk 0, stagger
                # the k and v loads behind the previous tensor's first MMs
                # so the first matmul inputs aren't stuck behind 9MB of DMA
                for name_, w_sb, b_sb, dsts in (
                    ("q", wq_sb, bq_sb, qT),
                    ("k", wk_sb, bk_sb, kT),
                ):
                    src = xq if name_ == "q" else xk
                    gate = None
                    if c == 0:
                        gate = gates.get("q" if name_ == "k" else None)
                    xcs = load_xc(src, c, gate)
                    if name_ == "k" and c == 0:
                        for kd in range(8):
                            d = nc.sync.dma_start(wk_sb[:, kd, :],
                                                  wk[:, kd, :])
                    for eb in range(2):
                        pps = ppp.tile([128, CH], F32, tag="pp", name="pp")
                        for kd in range(8):
                            mm = nc.tensor.matmul(
                                pps[:],
                                w_sb[:, kd, eb * 128:(eb + 1) * 128],
                                xcs(kd),
                                start=(kd == 0), stop=(kd == 7))
                            if eb == 0 and kd == 0:
                                gates[name_] = mm
                        nc.vector.tensor_scalar_add(
                            dsts[eb][:, csl], pps[:], b_sb[:, eb:eb + 1])

                # v projection -> va[t, e] with ones columns at 64/129
                if c == 0:
                    for kd in range(8):
                        d = nc.sync.dma_start(wv_sb[:, kd, :], wv[:, kd, :])
                        add_dep_helper(d.ins, gates["q"].ins, sync=True,
                                       reason="dma-throttle")
                xcs = load_xc(xv, c, gates.get("k") if c == 0 else None)
                for j in range(4):
                    tb = 4 * c + j
                    vps = ppp.tile([128, E], F32, tag="pp", name="pp")
                    for kd in range(8):
                        nc.tensor.matmul(
                            vps[:],
                            xcs(kd)[:, j * 128:(j + 1) * 128],
                            wv_sb[:, kd, :],
                            start=(kd == 0), stop=(kd == 7))
                    for p in range(2):
                        for hh in range(2):
                            e0 = 128 * p + 64 * hh
                            nc.vector.tensor_add(
                                va[p][:, tb, 65 * hh:65 * hh + 64],
                                vps[:, e0:e0 + 64], bvb[:, e0:e0 + 64])

            def emit_attn(c):
                csl = slice(c * CH, (c + 1) * CH)
                # attention for this chunk, one head-pair at a time;
                # score/exp and ctx matmuls grouped 4 tk-tiles at a time
                n_tkb = 4 * (c + 1)
                for p in range(2):
                    cps = [ctxp.tile([65, CH], F32, tag=f"ctx{hh}",
                                     name=f"ctx{hh}") for hh in range(2)]
                    ets_group = {}
                    for g in range(c + 1):
                        for tkb in range(4 * g, 4 * g + 4):
                            etps = etpp.tile([128, 2, CH], F32, tag="et",
                                             name="et")
                            for hh in range(2):
                                nc.tensor.matmul(
                                    etps[:, hh, :],
                                    kT[p][64 * hh:64 * (hh + 1),
                                          tkb * 128:(tkb + 1) * 128],
                                    qT[p][64 * hh:64 * (hh + 1), csl],
                                    start=True, stop=True,
                                    tile_position=(64 * hh, 0))
                            ets = wkp.tile([128, 2, CH], F32R, tag="ets",
                                           name="ets", bufs=6)
                            nc.scalar.activation(ets[:], etps[:], EXP,
                                                 scale=0.125)
                            if g == c:
                                # diagonal block: causal mask, head 0 on
                                # gpsimd, head 1 on DVE (parallel engines)
                                r = tkb - 4 * c
                                nc.gpsimd.affine_select(
                                    out=ets[:, 0, :], in_=ets[:, 0, :],
                                    pattern=[[1, CH]], base=-r * 128,
                                    channel_multiplier=-1,
                                    compare_op=mybir.AluOpType.is_ge,
                                    fill=0.0)
                                nc.vector.tensor_mul(
                                    ets[:, 1, :], ets[:, 1, :],
                                    masks[:, r, :])
                            ets_group[tkb] = ets
                        for tkb in range(4 * g, 4 * g + 4):
                            ets = ets_group.pop(tkb)
                            for hh in range(2):
                                nc.tensor.matmul(
                                    cps[hh][:],
                                    va[p][:, tkb, 65 * hh:65 * (hh + 1)],
                                    ets[:, hh, :],
                                    start=(tkb == 0), stop=(tkb == n_tkb - 1))
                    # softmax denominators: single-op approx reciprocal
                    # (~18 bits), then broadcast across partitions
                    for hh in range(2):
                        zrow = wkp.tile([1, CH], F32, tag="zrow", name="zrow",
                                        bufs=2)
                        nc.vector.tensor_copy(zrow[:], cps[hh][64:65, :])
                        zrec = wkp.tile([1, CH], F32, tag="zrec", name="zrec",
                                        bufs=2)
                        nc.vector.reciprocal_approx_fast(zrec[:], zrow[:])
                        zbh = wkp.tile([64, CH], F32, tag="zbh", name="zbh",
                                       bufs=2)
                        nc.gpsimd.partition_broadcast(zbh[:], zrec[:],
                                                      channels=64)
                        if hh == 0:
                            nc.vector.tensor_mul(ctxT[p][0:64, csl],
                                                 cps[0][0:64, :], zbh[:])
                        else:
                            ctmp = wkp.tile([64, CH], F32R, tag="ctmp",
                                            name="ctmp", bufs=2)
                            nc.vector.tensor_mul(ctmp[:], cps[1][0:64, :],
                                                 zbh[:])
                            nc.sync.dma_start(ctxT[p][64:128, csl], ctmp[:])

            def emit_outproj(c):
                # output projection for this chunk's 4 token blocks
                for j in range(4):
                    tb = 4 * c + j
                    for nb in range(2):
                        ops = ppp.tile([128, CH], F32, tag="pp", name="pp")
                        for p in range(2):
                            nc.tensor.matmul(
                                ops[:],
                                ctxT[p][:, tb * 128:(tb + 1) * 128],
                                wo_sb[:, p, nb * CH:(nb + 1) * CH],
                                start=(p == 0), stop=(p == 1))
                        osb = wkp.tile([128, CH], F32, tag="osb", name="osb",
                                       bufs=3)
                        nc.vector.tensor_copy(osb[:], ops[:])
                        nc.sync.dma_start(
                            part[tb * 128:(tb + 1) * 128,
                                 nb * CH:(nb + 1) * CH], osb[:])

            emit_proj(0)
            nc.sync.dma_start(wo_sb[:], wo[:])
            for c in range(N_CH):
                emit_attn(c)
                if c + 1 < N_CH:
                    emit_proj(c + 1)
                emit_outproj(c)
    nc.compile()
    return nc


def _get_nc():
    global _NC_CACHE
    if _NC_CACHE is None:
        _NC_CACHE = build_nc()
    return _NC_CACHE


def _pack_x(xb):
    # [S, D_MODEL] -> [128, N_CH, 8, CH]:  out[p, c, kd, t] = x[c*CH+t, kd*128+p]
    xT = xb.T.reshape(8, 128, N_CH, CH)
    return np.ascontiguousarray(xT.transpose(1, 2, 0, 3))


def _pack_w(w):
    # [E_rows, D_MODEL] slice transposed -> [128, 8, E]
    wT = w.T.reshape(8, 128, w.shape[0])
    return np.ascontiguousarray(wT.transpose(1, 0, 2))


def make_in_maps(query, key, value, Wq, bq, Wk, bk, Wv, bv, Wo):
    query = np.asarray(query, dtype=np.float32)
    key = np.asarray(key, dtype=np.float32)
    value = np.asarray(value, dtype=np.float32)
    in_maps = []
    for core in range(N_CORES):
        b = core // 4
        hg = core % 4
        e0 = hg * E
        esl = slice(e0, e0 + E)
        wo_c = np.asarray(Wo, np.float32)[:, esl].T  # [E, D_MODEL]
        m = {
            "xq": _pack_x(query[b]),
            "xk": _pack_x(key[b]),
            "xv": _pack_x(value[b]),
            "wq": _pack_w(np.asarray(Wq, np.float32)[esl, :]),
            "wk": _pack_w(np.asarray(Wk, np.float32)[esl, :]),
            "wv": _pack_w(np.asarray(Wv, np.float32)[esl, :]),
            "wo": np.ascontiguousarray(
                wo_c.reshape(2, 128, D_MODEL).transpose(1, 0, 2)),
            "bq": np.ascontiguousarray(
                np.asarray(bq, np.float32)[esl].reshape(2, 128).T),
            "bk": np.ascontiguousarray(
                np.asarray(bk, np.float32)[esl].reshape(2, 128).T),
            "bv": np.ascontiguousarray(
                np.asarray(bv, np.float32)[esl].reshape(1, E)),
        }
        in_maps.append(m)
    return in_maps


def run(inputs, trace=False):
    nc = _get_nc()
    in_maps = make_in_maps(
        inputs["query"], inputs["key"], inputs["value"],
        inputs["Wq"], inputs["bq"], inputs["Wk"], inputs["bk"],
        inputs["Wv"], inputs["bv"], inputs["Wo"])
    res = run_bass_kernel_spmd(nc, in_maps, core_ids=list(range(N_CORES)),
                               trace=trace)
    bo = np.asarray(inputs["bo"], np.float32)
    out = np.zeros((B, S, D_MODEL), np.float32)
    for core in range(N_CORES):
        out[core // 4] += res.results[core]["part"]
    out += bo[None, None, :]
    return out, res


def kernel(**inputs) -> np.ndarray:
    out, _ = run(inputs, trace=False)
    return out
